# revision 21
# baseline (speedup 1.0000x reference)
"""Multi-head causal self-attention (B=2, S=2048, D=2048, H=16) on 8 trn2
NeuronCores.

Sharding: tensor-parallel over heads. Core c owns heads {2c, 2c+1}:
  - QKV projection for its 2 heads (contraction over the full d_model),
  - causal attention for its 2 heads,
  - partial output projection  O_c = A_c @ W_out[:, c*256:(c+1)*256].T
Host sums the 8 partial outputs (the "all-reduce after out_proj" of the
TP scheme, done on host since the full output is assembled there anyway).

All on-device compute is laid out "feature-major" (transposed) so no
transposes are ever needed. Everything is bf16 except the PSUM (f32)
and the normalizer reciprocal: bf16 keeps every matmul at full PE rate
regardless of free-dim, enables fast weight load on all stationaries,
and halves DMA/SBUF traffic.

The attention inner loop is co-limited by the PE and the ACT engine
(exp costs (N+352)/1.2 ns per instruction), so scores are computed in
PAIRS of 128-key tiles into a two-bank [128, 1024] PSUM tile and a
single exp covers both: 573 ns/k-tile instead of 720. Three such score
tiles rotate (6 banks); attn and the normalizer each hold one bank and
are drained to SBUF at block end so bufs=1 suffices. The causal mask is
applied inside the scores accumulation group via one extra 128-row
matmul (adds -1e9 to the k > q triangle), keeping the exp -> AV chain
entirely off the vector engine.

Structure per batch (attention interleaved with QKV by chunk):
  for j: QKV(chunk j) -> attn(j,h0) -> fin(prev) -> attn(j,h1)
         -> fin(j,h0) -> outproj(j-1)
AV lags scores by 2 groups (4 k-tiles), normalization lags one block,
out-proj two blocks: every PE wait target is produced ~2.5 us earlier.
Output stores ride the gpsimd queue so input prefetch is never stuck
behind stores on the sync queue.
"""

import math

import ml_dtypes
import numpy as np

import concourse.bass as bass
import concourse.tile as tile
from concourse import bacc, mybir
from concourse.bass_utils import run_bass_kernel_spmd

F32 = mybir.dt.float32
F32R = mybir.dt.float32r
BF16 = mybir.dt.bfloat16

N_CORES = 8


class Cfg:
    def __init__(self, B=2, S=2048, D=2048, n_heads=16):
        self.B = B
        self.S = S
        self.D = D
        self.n_heads = n_heads
        self.Dh = 128
        self.DHT = n_heads * self.Dh       # W_qkv section stride (q/k/v)
        self.HPC = n_heads // N_CORES      # heads per core (2)
        self.QC = 512                      # token chunk (matmul free dim)
        self.KT = D // 128                 # k-tiles over d_model
        self.NCH = S // self.QC            # token chunks per batch
        assert self.HPC == 2 and D % 128 == 0 and S % self.QC == 0


def build_kernel(cfg: Cfg):
    """Build the SPMD single-core program. Returns compiled nc."""
    B, S, D, QC, KT, NCH = cfg.B, cfg.S, cfg.D, cfg.QC, cfg.KT, cfg.NCH
    Dh = cfg.Dh
    NQT = QC // 128                      # 128-token subtiles per chunk
    inv_sqrt_dh = 1.0 / math.sqrt(Dh)

    nc = bacc.Bacc("TRN2", target_bir_lowering=False, debug=False,
                   num_devices=N_CORES)

    xT = nc.dram_tensor("xT", [D, B * S], BF16, kind="ExternalInput").ap()
    wqkvT = nc.dram_tensor("wqkvT", [D, 768], BF16, kind="ExternalInput").ap()
    woutT = nc.dram_tensor("woutT", [256, D], BF16, kind="ExternalInput").ap()
    ident = nc.dram_tensor("ident", [128, 128], BF16,
                           kind="ExternalInput").ap()
    tri = nc.dram_tensor("tri", [128, 128], BF16, kind="ExternalInput").ap()
    ones_col = nc.dram_tensor("ones_col", [128, 1], BF16,
                              kind="ExternalInput").ap()
    ones_row = nc.dram_tensor("ones_row", [1, 128], BF16,
                              kind="ExternalInput").ap()
    outT = nc.dram_tensor("outT", [D, B * S], BF16, kind="ExternalOutput").ap()
    # h1 partial of the very last chunk's out-proj (host adds it into the
    # last QC columns) — lets the h0 half run a block earlier, off the tail
    outT2 = nc.dram_tensor("outT2", [D, QC], BF16, kind="ExternalOutput").ap()

    with tile.TileContext(nc) as tc:
        with (
            tc.tile_pool(name="wpool", bufs=1) as wpool,
            tc.tile_pool(name="xpool", bufs=9) as xpool,
            tc.tile_pool(name="qkvpool", bufs=1) as qkvpool,
            tc.tile_pool(name="apool", bufs=6) as apool,
            tc.tile_pool(name="ppool", bufs=12) as ppool,
            # attn_sb / rb_sb have block-long lifetimes; a separate pool
            # keeps their slot reuse from WAR-stalling the exp pipeline
            # (ppool slots recycle within ~a block)
            tc.tile_pool(name="fpool", bufs=8) as fpool,
            tc.tile_pool(name="opool", bufs=6) as opool,
            tc.tile_pool(name="smallpool", bufs=4) as smallpool,
            # 3 x [128,1024] two-bank score tiles; also serves QKV,
            # rank-1 and out-proj psums (sequential on the PE timeline)
            tc.tile_pool(name="scoreps", bufs=3, space="PSUM") as scoreps,
            tc.tile_pool(name="attnps", bufs=1, space="PSUM") as attnps,
            tc.tile_pool(name="rps", bufs=1, space="PSUM") as rps,
        ):
            HKT = KT // 4

            def load_x_chunk(b, j):
                """Four quarter DMAs for one 512-token chunk of x^T."""
                col0 = b * S + j * QC
                halves = []
                for hh in range(4):
                    t = xpool.tile([128, HKT * QC], BF16, tag="xt", name="xt")
                    src = xT[hh * HKT * 128:(hh + 1) * HKT * 128,
                             col0:col0 + QC]
                    nc.sync.dma_start(
                        t[:].rearrange("p (k c) -> p k c", k=HKT),
                        src.rearrange("(k p) c -> p k c", p=128))
                    halves.append(t)
                return halves

            # ---- static weights / constants, interleaved with the first
            # x chunk in exactly the k-loop consumption order (w k-quad,
            # then the x quarter that pairs with it) so the first QKV
            # psum group never waits on a later DMA ----
            w_tiles = [None] * KT
            first_chunk = []
            col0 = 0
            for hh in range(4):
                for k in range(hh * HKT, (hh + 1) * HKT):
                    t = wpool.tile([128, 768], BF16, tag=f"w{k}",
                                   name=f"w{k}")
                    nc.sync.dma_start(t[:], wqkvT[k * 128:(k + 1) * 128, :])
                    w_tiles[k] = t
                t = xpool.tile([128, HKT * QC], BF16, tag="xt", name="xt")
                src = xT[hh * HKT * 128:(hh + 1) * HKT * 128, col0:col0 + QC]
                nc.sync.dma_start(
                    t[:].rearrange("p (k c) -> p k c", k=HKT),
                    src.rearrange("(k p) c -> p k c", p=128))
                first_chunk.append(t)
            onec_t = wpool.tile([128, 1], BF16, tag="onec", name="onec")
            nc.sync.dma_start(onec_t[:], ones_col[:])
            oner_t = wpool.tile([1, 128], BF16, tag="oner", name="oner")
            nc.sync.dma_start(oner_t[:], ones_row[:])
            ident_t = wpool.tile([128, 128], BF16, tag="ident", name="ident")
            nc.sync.dma_start(ident_t[:], ident[:])
            tri_t = wpool.tile([128, 128], BF16, tag="tri", name="tri")
            nc.sync.dma_start(tri_t[:], tri[:])
            wo_tiles = []
            for hh in range(2):
                t = wpool.tile([128, D], BF16, tag=f"wo{hh}", name=f"wo{hh}")
                nc.sync.dma_start(t[:], woutT[hh * 128:(hh + 1) * 128, :])
                wo_tiles.append(t)

            # x chunk prefetch: issued one chunk ahead, at the START of
            # the previous chunk's emission, so the load is never queued
            # behind output stores on the sync queue
            x_pending = {(0, 0): first_chunk}

            for b in range(B):
                # ---- persistent per-batch QKV / A tiles ----
                # comps: 0=Q_h0 1=K_h0 2=Q_h1 3=K_h1 (dh-major [128, S])
                qk_sb = [qkvpool.tile([128, S], BF16, tag=f"qk{c}", name=f"qk{c}")
                         for c in range(4)]
                # V token-major: tile per 128 tokens, [128, 256] (2 heads)
                v_sb = [qkvpool.tile([128, 256], BF16, tag=f"v{t}", name=f"v{t}")
                        for t in range(S // 128)]
                # A^T per (chunk, head) [128, QC] — per-chunk tiles so
                # out-proj(j) depends only on its own chunk's writes (a
                # whole-S tile would serialize on the NEXT chunk's mul
                # via tile-granularity dependency tracking)
                a_sb = {}

                def emit_qkv_chunk(j):
                    halves = x_pending.pop((b, j))
                    nxt = (b, j + 1) if j + 1 < NCH else (b + 1, 0)
                    if nxt[0] < B:
                        x_pending[nxt] = load_x_chunk(*nxt)

                    def xt_sl(k, f0, f1):
                        t = halves[k // HKT]
                        kk = k % HKT
                        return t[:, kk * QC + f0: kk * QC + f1]

                    # Q^T / K^T for both heads (copies on DVE: the ACT
                    # engine is reserved for exp, which must never fall
                    # behind the attention inner loop)
                    for c in range(4):
                        ps = scoreps.tile([128, QC], F32, tag="ps", name="ps")
                        for k in range(KT):
                            nc.tensor.matmul(
                                ps[:],
                                (w_tiles[k][:, c * 128:(c + 1) * 128]),
                                (xt_sl(k, 0, QC)),
                                start=(k == 0), stop=(k == KT - 1))
                        nc.vector.tensor_copy(
                            qk_sb[c][:, j * QC:(j + 1) * QC], ps[:])
                    # V token-major (both heads side by side)
                    for sub in range(NQT):
                        ps = scoreps.tile([128, 256], F32, tag="ps", name="ps")
                        for k in range(KT):
                            nc.tensor.matmul(
                                ps[:],
                                (xt_sl(k, sub * 128, (sub + 1) * 128)),
                                (w_tiles[k][:, 512:768]),
                                start=(k == 0), stop=(k == KT - 1))
                        nc.vector.tensor_copy(v_sb[j * NQT + sub][:], ps[:])

                def emit_attn_block(j, h):
                    # scores in groups of k-tiles -> one exp per group; AV/r
                    # lag by GSKEW groups so the PSUM->exp latency stays off
                    # the PE's in-order path. j=0 blocks use single-kt
                    # groups so the lag still covers the exp latency.
                    GSKEW = 3
                    n_kt = (j + 1) * QC // 128
                    gsz = 1 if n_kt <= 2 * GSKEW else 2
                    kt_groups = [list(range(g, min(g + gsz, n_kt)))
                                 for g in range(0, n_kt, gsz)]
                    n_g = len(kt_groups)
                    qT = qk_sb[2 * h]
                    kTl = qk_sb[2 * h + 1]
                    attn = attnps.tile([128, QC], F32, tag="attn",
                                       name="attn")
                    r = rps.tile([1, QC], F32, tag="r", name="r")
                    groups = {}

                    def emit_scores_group(g):
                        kts = kt_groups[g]
                        s_ps = scoreps.tile([128, len(kts) * QC], F32,
                                            tag="ps", name="ps")
                        f_avs = []
                        for i, kt in enumerate(kts):
                            rel = kt * 128 - j * QC
                            f_av = max(rel, 0)
                            diag = rel >= 0
                            base = i * QC
                            nc.tensor.matmul(
                                s_ps[:, base + f_av:base + QC],
                                kTl[:, kt * 128:(kt + 1) * 128],
                                qT[:, j * QC + f_av:(j + 1) * QC],
                                start=True, stop=not diag)
                            if diag:
                                # add -1e9 to the k > q triangle (I.T @ tri)
                                # inside the accumulation group
                                nc.tensor.matmul(
                                    s_ps[:, base + rel:base + rel + 128],
                                    ident_t[:], tri_t[:],
                                    start=False, stop=True)
                            f_avs.append(f_av)
                        p_sb = ppool.tile([128, len(kts) * QC], BF16,
                                          tag="p", name="p")
                        # one exp for the group; leading trim only (the
                        # skipped region is never read downstream)
                        f0 = f_avs[0]
                        nc.scalar.activation(
                            p_sb[:, f0:], s_ps[:, f0:],
                            mybir.ActivationFunctionType.Exp,
                            scale=inv_sqrt_dh)
                        groups[g] = (p_sb, f_avs)

                    def emit_av_group(g):
                        p_sb, f_avs = groups.pop(g)
                        for i, kt in enumerate(kt_groups[g]):
                            f_av = f_avs[i]
                            base = i * QC
                            nc.tensor.matmul(
                                attn[:, f_av:],
                                v_sb[kt][:, h * 128:(h + 1) * 128],
                                p_sb[:, base + f_av:base + QC],
                                start=(kt == 0), stop=(kt == n_kt - 1))
                            nc.tensor.matmul(
                                r[:, f_av:], onec_t[:],
                                p_sb[:, base + f_av:base + QC],
                                start=(kt == 0), stop=(kt == n_kt - 1))

                    for g in range(n_g):
                        emit_scores_group(g)
                        if g >= GSKEW:
                            emit_av_group(g - GSKEW)
                    for g in range(max(0, n_g - GSKEW), n_g):
                        emit_av_group(g)
                    # drain attn psum to SBUF now (frees the single attn
                    # bank) and launch the reciprocal (DVE); both consumed
                    # one block later
                    attn_sb = fpool.tile([128, QC], F32R, tag="f", name="f")
                    nc.vector.tensor_copy(attn_sb[:], attn[:])
                    recip = smallpool.tile([1, QC], F32, tag="recip",
                                           name="recip")
                    nc.vector.reciprocal_approx_fast(recip[:], r[:])
                    recip_b = smallpool.tile([1, QC], BF16, tag="recipb",
                                             name="recipb")
                    nc.vector.tensor_copy(recip_b[:], recip[:])
                    return (j, h, attn_sb, recip_b)

                def emit_finalize(blk):
                    j, h, attn_sb, recip_b = blk
                    rb_ps = scoreps.tile([128, QC], F32, tag="ps", name="ps")
                    nc.tensor.matmul(rb_ps[:], oner_t[:], recip_b[:],
                                     start=True, stop=True)
                    rb_sb = fpool.tile([128, QC], F32R, tag="f", name="f")
                    nc.vector.tensor_copy(rb_sb[:], rb_ps[:])
                    at = apool.tile([128, QC], BF16, tag="a", name="a")
                    nc.vector.tensor_mul(at[:], attn_sb[:], rb_sb[:])
                    a_sb[(j, h)] = at

                def emit_outproj_head(j, h, dst, qoff):
                    # single-head tail half: drains alternate ACT/DVE (no
                    # exps remain near the tail), stores on both queues
                    for m in range(D // 128):
                        ps = scoreps.tile([128, QC], F32, tag="ps", name="ps")
                        nc.tensor.matmul(
                            ps[:],
                            wo_tiles[h][:, m * 128:(m + 1) * 128],
                            a_sb[(j, h)][:],
                            start=True, stop=True)
                        o_sb = opool.tile([128, QC], BF16, tag="o", name="o")
                        if m % 2 == 1:
                            nc.scalar.copy(o_sb[:], ps[:])
                        else:
                            nc.vector.tensor_copy(o_sb[:], ps[:])
                        q = nc.gpsimd if m % 2 == 0 else nc.sync
                        q.dma_start(
                            dst[m * 128:(m + 1) * 128, qoff:qoff + QC],
                            o_sb[:])

                def emit_outproj(j, last=False):
                    # partial over this core's 256 head-features; drains on
                    # DVE (ACT is saturated by exp — except on the final
                    # chunk, where no exps remain and alternating halves
                    # the drain tail); stores alternate between the two
                    # otherwise-idle DMA queues.
                    col0 = b * S + j * QC
                    for m in range(D // 128):
                        ps = scoreps.tile([128, QC], F32, tag="ps", name="ps")
                        for h in range(2):
                            nc.tensor.matmul(
                                ps[:],
                                wo_tiles[h][:, m * 128:(m + 1) * 128],
                                a_sb[(j, h)][:],
                                start=(h == 0), stop=(h == 1))
                        o_sb = opool.tile([128, QC], BF16, tag="o", name="o")
                        if last and m % 2 == 1:
                            nc.scalar.copy(o_sb[:], ps[:])
                        else:
                            nc.vector.tensor_copy(o_sb[:], ps[:])
                        q = nc.gpsimd if m % 2 == 0 else nc.sync
                        q.dma_start(
                            outT[m * 128:(m + 1) * 128, col0:col0 + QC],
                            o_sb[:])

                blocks = []
                for j in range(NCH):
                    emit_qkv_chunk(j)
                    for h in range(2):
                        blocks.append(emit_attn_block(j, h))
                        n = len(blocks)
                        if n >= 2:
                            emit_finalize(blocks[n - 2])
                        if n >= 3 and blocks[n - 3][1] == 1:
                            emit_outproj(blocks[n - 3][0])
                emit_finalize(blocks[-1])
                if b == B - 1:
                    # split the final chunk's out-proj by head: h0 depends
                    # only on finalize(j,h0) and runs before the h1 chain,
                    # halving the post-attention tail
                    jl = blocks[-1][0]
                    emit_outproj_head(jl, 0, outT, b * S + jl * QC)
                    emit_outproj_head(jl, 1, outT2, 0)
                else:
                    emit_outproj(blocks[-1][0])

    nc.compile()
    return nc


def make_inputs(cfg: Cfg, x, W_qkv, W_out):
    """Host-side sharding: returns in_maps (list of 8 dicts)."""
    B, S, D = cfg.B, cfg.S, cfg.D
    Dh = cfg.Dh
    xTa = np.ascontiguousarray(
        x.reshape(B * S, D).T).astype(ml_dtypes.bfloat16)  # [D, B*S]

    p = np.arange(128)[:, None]
    c = np.arange(128)[None, :]
    tri = np.where(p > c, -1e9, 0.0).astype(ml_dtypes.bfloat16)
    identm = np.eye(128, dtype=ml_dtypes.bfloat16)
    ones_col = np.ones((128, 1), dtype=ml_dtypes.bfloat16)
    ones_row = np.ones((1, 128), dtype=ml_dtypes.bfloat16)

    in_maps = []
    DHT = cfg.DHT
    for cidx in range(N_CORES):
        h0 = cfg.HPC * cidx
        wq = np.empty((D, 768), dtype=np.float32)          # [D, cols]
        for i, h in enumerate((h0, h0 + 1)):
            wq[:, (2 * i) * 128:(2 * i) * 128 + 128] = \
                W_qkv[0 * DHT + h * Dh: 0 * DHT + h * Dh + Dh, :].T   # Q_h
            wq[:, (2 * i + 1) * 128:(2 * i + 1) * 128 + 128] = \
                W_qkv[1 * DHT + h * Dh: 1 * DHT + h * Dh + Dh, :].T   # K_h
            wq[:, 512 + i * 128: 512 + (i + 1) * 128] = \
                W_qkv[2 * DHT + h * Dh: 2 * DHT + h * Dh + Dh, :].T   # V_h
        wo = np.ascontiguousarray(
            W_out[:, h0 * Dh:(h0 + cfg.HPC) * Dh].T).astype(
                ml_dtypes.bfloat16)
        in_maps.append({
            "xT": xTa,
            "wqkvT": wq.astype(ml_dtypes.bfloat16),
            "woutT": wo,
            "ident": identm,
            "tri": tri,
            "ones_col": ones_col,
            "ones_row": ones_row,
        })
    return in_maps


_CACHED = {}


def kernel(x, W_qkv, W_out, mask=None, **_ignored):
    cfg = Cfg(B=x.shape[0], S=x.shape[1], D=x.shape[2],
              n_heads=W_qkv.shape[0] // 384)
    key = (cfg.B, cfg.S, cfg.D)
    if key not in _CACHED:
        _CACHED[key] = build_kernel(cfg)
    nc = _CACHED[key]
    in_maps = make_inputs(cfg, np.asarray(x), np.asarray(W_qkv),
                          np.asarray(W_out))
    res = run_bass_kernel_spmd(nc, in_maps, list(range(N_CORES)))
    acc = res.results[0]["outT"].astype(np.float32)
    acc2 = res.results[0]["outT2"].astype(np.float32)
    for c in range(1, N_CORES):
        acc = acc + res.results[c]["outT"].astype(np.float32)
        acc2 = acc2 + res.results[c]["outT2"].astype(np.float32)
    acc[:, cfg.B * cfg.S - cfg.QC:] += acc2
    out = acc.T.reshape(cfg.B, cfg.S, cfg.D)
    return np.ascontiguousarray(out)


# revision 23
# speedup vs baseline: 1.0783x; 1.0783x over previous
"""Multi-head causal self-attention (B=2, S=2048, D=2048, H=16) on 8 trn2
NeuronCores.

Sharding: tensor-parallel over heads. Core c owns heads {2c, 2c+1}:
  - QKV projection for its 2 heads (contraction over the full d_model),
  - causal attention for its 2 heads,
  - partial output projection  O_c = A_c @ W_out[:, c*256:(c+1)*256].T
Host sums the 8 partial outputs (the "all-reduce after out_proj" of the
TP scheme, done on host since the full output is assembled there anyway).

All on-device compute is laid out "feature-major" (transposed) so no
transposes are ever needed. Everything is bf16 except the PSUM (f32)
and the normalizer reciprocal: bf16 keeps every matmul at full PE rate
regardless of free-dim, enables fast weight load on all stationaries,
and halves DMA/SBUF traffic.

The attention inner loop is co-limited by the PE and the ACT engine
(exp costs (N+352)/1.2 ns per instruction), so scores are computed in
PAIRS of 128-key tiles into a two-bank [128, 1024] PSUM tile and a
single exp covers both: 573 ns/k-tile instead of 720. Three such score
tiles rotate (6 banks); attn and the normalizer each hold one bank and
are drained to SBUF at block end so bufs=1 suffices. The causal mask is
applied inside the scores accumulation group via one extra 128-row
matmul (adds -1e9 to the k > q triangle), keeping the exp -> AV chain
entirely off the vector engine.

Structure per batch (attention interleaved with QKV by chunk):
  for j: QKV(chunk j) -> attn(j,h0) -> fin(prev) -> attn(j,h1)
         -> fin(j,h0) -> outproj(j-1)
AV lags scores by 2 groups (4 k-tiles), normalization lags one block,
out-proj two blocks: every PE wait target is produced ~2.5 us earlier.
Output stores ride the gpsimd queue so input prefetch is never stuck
behind stores on the sync queue.
"""

import math

import ml_dtypes
import numpy as np

import concourse.bass as bass
import concourse.tile as tile
from concourse import bacc, mybir
from concourse.bass_utils import run_bass_kernel_spmd

F32 = mybir.dt.float32
F32R = mybir.dt.float32r
BF16 = mybir.dt.bfloat16

N_CORES = 8


class Cfg:
    def __init__(self, B=2, S=2048, D=2048, n_heads=16):
        self.B = B
        self.S = S
        self.D = D
        self.n_heads = n_heads
        self.Dh = 128
        self.DHT = n_heads * self.Dh       # W_qkv section stride (q/k/v)
        self.HPC = n_heads // N_CORES      # heads per core (2)
        self.QC = 512                      # token chunk (matmul free dim)
        self.KT = D // 128                 # k-tiles over d_model
        self.NCH = S // self.QC            # token chunks per batch
        assert self.HPC == 2 and D % 128 == 0 and S % self.QC == 0


def build_kernel(cfg: Cfg):
    """Build the SPMD single-core program. Returns compiled nc."""
    B, S, D, QC, KT, NCH = cfg.B, cfg.S, cfg.D, cfg.QC, cfg.KT, cfg.NCH
    Dh = cfg.Dh
    NQT = QC // 128                      # 128-token subtiles per chunk
    inv_sqrt_dh = 1.0 / math.sqrt(Dh)

    nc = bacc.Bacc("TRN2", target_bir_lowering=False, debug=False,
                   num_devices=N_CORES)

    xT = nc.dram_tensor("xT", [D, B * S], BF16, kind="ExternalInput").ap()
    wqkvT = nc.dram_tensor("wqkvT", [D, 768], BF16, kind="ExternalInput").ap()
    woutT = nc.dram_tensor("woutT", [256, D], BF16, kind="ExternalInput").ap()
    ident = nc.dram_tensor("ident", [128, 128], BF16,
                           kind="ExternalInput").ap()
    tri = nc.dram_tensor("tri", [128, 128], BF16, kind="ExternalInput").ap()
    ones_col = nc.dram_tensor("ones_col", [128, 1], BF16,
                              kind="ExternalInput").ap()
    ones_row = nc.dram_tensor("ones_row", [1, 128], BF16,
                              kind="ExternalInput").ap()
    outT = nc.dram_tensor("outT", [D, B * S], BF16, kind="ExternalOutput").ap()
    # h1 partial of the very last chunk's out-proj (host adds it into the
    # last QC columns) — lets the h0 half run a block earlier, off the tail
    outT2 = nc.dram_tensor("outT2", [D, QC], BF16, kind="ExternalOutput").ap()

    with tile.TileContext(nc) as tc:
        with (
            tc.tile_pool(name="wpool", bufs=1) as wpool,
            tc.tile_pool(name="xpool", bufs=9) as xpool,
            tc.tile_pool(name="qkvpool", bufs=1) as qkvpool,
            tc.tile_pool(name="apool", bufs=6) as apool,
            tc.tile_pool(name="ppool", bufs=12) as ppool,
            # attn_sb / rb_sb have block-long lifetimes; a separate pool
            # keeps their slot reuse from WAR-stalling the exp pipeline
            # (ppool slots recycle within ~a block)
            tc.tile_pool(name="fpool", bufs=8) as fpool,
            tc.tile_pool(name="opool", bufs=6) as opool,
            tc.tile_pool(name="smallpool", bufs=4) as smallpool,
            # 3 x [128,1024] two-bank score tiles; also serves QKV,
            # rank-1 and out-proj psums (sequential on the PE timeline)
            tc.tile_pool(name="scoreps", bufs=3, space="PSUM") as scoreps,
            tc.tile_pool(name="attnps", bufs=1, space="PSUM") as attnps,
            tc.tile_pool(name="rps", bufs=1, space="PSUM") as rps,
        ):
            HKT = KT // 4

            def load_x_chunk(b, j):
                """Four quarter DMAs for one 512-token chunk of x^T."""
                col0 = b * S + j * QC
                halves = []
                for hh in range(4):
                    t = xpool.tile([128, HKT * QC], BF16, tag="xt", name="xt")
                    src = xT[hh * HKT * 128:(hh + 1) * HKT * 128,
                             col0:col0 + QC]
                    nc.sync.dma_start(
                        t[:].rearrange("p (k c) -> p k c", k=HKT),
                        src.rearrange("(k p) c -> p k c", p=128))
                    halves.append(t)
                return halves

            # ---- static weights / constants, interleaved with the first
            # x chunk in exactly the k-loop consumption order (w k-quad,
            # then the x quarter that pairs with it) so the first QKV
            # psum group never waits on a later DMA ----
            w_tiles = [None] * KT
            first_chunk = []
            col0 = 0
            for hh in range(4):
                for k in range(hh * HKT, (hh + 1) * HKT):
                    t = wpool.tile([128, 768], BF16, tag=f"w{k}",
                                   name=f"w{k}")
                    nc.sync.dma_start(t[:], wqkvT[k * 128:(k + 1) * 128, :])
                    w_tiles[k] = t
                t = xpool.tile([128, HKT * QC], BF16, tag="xt", name="xt")
                src = xT[hh * HKT * 128:(hh + 1) * HKT * 128, col0:col0 + QC]
                nc.sync.dma_start(
                    t[:].rearrange("p (k c) -> p k c", k=HKT),
                    src.rearrange("(k p) c -> p k c", p=128))
                first_chunk.append(t)
            onec_t = wpool.tile([128, 1], BF16, tag="onec", name="onec")
            nc.sync.dma_start(onec_t[:], ones_col[:])
            oner_t = wpool.tile([1, 128], BF16, tag="oner", name="oner")
            nc.sync.dma_start(oner_t[:], ones_row[:])
            ident_t = wpool.tile([128, 128], BF16, tag="ident", name="ident")
            nc.sync.dma_start(ident_t[:], ident[:])
            tri_t = wpool.tile([128, 128], BF16, tag="tri", name="tri")
            nc.sync.dma_start(tri_t[:], tri[:])
            wo_tiles = []
            for hh in range(2):
                t = wpool.tile([128, D], BF16, tag=f"wo{hh}", name=f"wo{hh}")
                nc.sync.dma_start(t[:], woutT[hh * 128:(hh + 1) * 128, :])
                wo_tiles.append(t)

            # x chunk prefetch: issued one chunk ahead, at the START of
            # the previous chunk's emission, so the load is never queued
            # behind output stores on the sync queue
            x_pending = {(0, 0): first_chunk}

            for b in range(B):
                # ---- persistent per-batch QKV / A tiles ----
                # comps: 0=Q_h0 1=K_h0 2=Q_h1 3=K_h1 (dh-major [128, S])
                qk_sb = [qkvpool.tile([128, S], BF16, tag=f"qk{c}", name=f"qk{c}")
                         for c in range(4)]
                # V token-major: tile per 128 tokens, [128, 256] (2 heads)
                v_sb = [qkvpool.tile([128, 256], BF16, tag=f"v{t}", name=f"v{t}")
                        for t in range(S // 128)]
                # A^T per (chunk, head) [128, QC] — per-chunk tiles so
                # out-proj(j) depends only on its own chunk's writes (a
                # whole-S tile would serialize on the NEXT chunk's mul
                # via tile-granularity dependency tracking)
                a_sb = {}

                def emit_qkv_chunk(j):
                    halves = x_pending.pop((b, j))
                    nxt = (b, j + 1) if j + 1 < NCH else (b + 1, 0)
                    if nxt[0] < B:
                        x_pending[nxt] = load_x_chunk(*nxt)

                    def xt_sl(k, f0, f1):
                        t = halves[k // HKT]
                        kk = k % HKT
                        return t[:, kk * QC + f0: kk * QC + f1]

                    # Q^T / K^T for both heads (copies on DVE: the ACT
                    # engine is reserved for exp, which must never fall
                    # behind the attention inner loop)
                    for c in range(4):
                        ps = scoreps.tile([128, QC], F32, tag="ps", name="ps")
                        for k in range(KT):
                            nc.tensor.matmul(
                                ps[:],
                                (w_tiles[k][:, c * 128:(c + 1) * 128]),
                                (xt_sl(k, 0, QC)),
                                start=(k == 0), stop=(k == KT - 1))
                        nc.vector.tensor_copy(
                            qk_sb[c][:, j * QC:(j + 1) * QC], ps[:])
                    # V token-major (both heads side by side)
                    for sub in range(NQT):
                        ps = scoreps.tile([128, 256], F32, tag="ps", name="ps")
                        for k in range(KT):
                            nc.tensor.matmul(
                                ps[:],
                                (xt_sl(k, sub * 128, (sub + 1) * 128)),
                                (w_tiles[k][:, 512:768]),
                                start=(k == 0), stop=(k == KT - 1))
                        nc.vector.tensor_copy(v_sb[j * NQT + sub][:], ps[:])

                def emit_attn_block(j, h):
                    # scores in groups of k-tiles -> one exp per group; AV/r
                    # lag by GSKEW groups so the PSUM->exp latency stays off
                    # the PE's in-order path. j=0 blocks use single-kt
                    # groups so the lag still covers the exp latency.
                    GSKEW = 3
                    n_kt = (j + 1) * QC // 128
                    gsz = 1 if n_kt <= 2 * GSKEW else 2
                    kt_groups = [list(range(g, min(g + gsz, n_kt)))
                                 for g in range(0, n_kt, gsz)]
                    n_g = len(kt_groups)
                    qT = qk_sb[2 * h]
                    kTl = qk_sb[2 * h + 1]
                    attn = attnps.tile([128, QC], F32, tag="attn",
                                       name="attn")
                    r = rps.tile([1, QC], F32, tag="r", name="r")
                    groups = {}

                    def emit_scores_group(g):
                        kts = kt_groups[g]
                        s_ps = scoreps.tile([128, len(kts) * QC], F32,
                                            tag="ps", name="ps")
                        f_avs = []
                        for i, kt in enumerate(kts):
                            rel = kt * 128 - j * QC
                            f_av = max(rel, 0)
                            diag = rel >= 0
                            base = i * QC
                            nc.tensor.matmul(
                                s_ps[:, base + f_av:base + QC],
                                kTl[:, kt * 128:(kt + 1) * 128],
                                qT[:, j * QC + f_av:(j + 1) * QC],
                                start=True, stop=not diag)
                            if diag:
                                # add -1e9 to the k > q triangle (I.T @ tri)
                                # inside the accumulation group
                                nc.tensor.matmul(
                                    s_ps[:, base + rel:base + rel + 128],
                                    ident_t[:], tri_t[:],
                                    start=False, stop=True)
                            f_avs.append(f_av)
                        p_sb = ppool.tile([128, len(kts) * QC], BF16,
                                          tag="p", name="p")
                        # one exp for the group; leading trim only (the
                        # skipped region is never read downstream)
                        f0 = f_avs[0]
                        nc.scalar.activation(
                            p_sb[:, f0:], s_ps[:, f0:],
                            mybir.ActivationFunctionType.Exp,
                            scale=inv_sqrt_dh)
                        groups[g] = (p_sb, f_avs)

                    def emit_av_group(g):
                        p_sb, f_avs = groups.pop(g)
                        for i, kt in enumerate(kt_groups[g]):
                            f_av = f_avs[i]
                            base = i * QC
                            nc.tensor.matmul(
                                attn[:, f_av:],
                                v_sb[kt][:, h * 128:(h + 1) * 128],
                                p_sb[:, base + f_av:base + QC],
                                start=(kt == 0), stop=(kt == n_kt - 1))
                            nc.tensor.matmul(
                                r[:, f_av:], onec_t[:],
                                p_sb[:, base + f_av:base + QC],
                                start=(kt == 0), stop=(kt == n_kt - 1))

                    for g in range(n_g):
                        emit_scores_group(g)
                        if g >= GSKEW:
                            emit_av_group(g - GSKEW)
                    for g in range(max(0, n_g - GSKEW), n_g):
                        emit_av_group(g)
                    # drain attn psum to SBUF now (frees the single attn
                    # bank) and launch the reciprocal (DVE); both consumed
                    # one block later
                    attn_sb = fpool.tile([128, QC], F32R, tag="f", name="f")
                    nc.vector.tensor_copy(attn_sb[:], attn[:])
                    recip = smallpool.tile([1, QC], F32, tag="recip",
                                           name="recip")
                    nc.vector.reciprocal_approx_fast(recip[:], r[:])
                    recip_b = smallpool.tile([1, QC], BF16, tag="recipb",
                                             name="recipb")
                    nc.vector.tensor_copy(recip_b[:], recip[:])
                    return (j, h, attn_sb, recip_b)

                def emit_finalize(blk):
                    j, h, attn_sb, recip_b = blk
                    rb_ps = scoreps.tile([128, QC], F32, tag="ps", name="ps")
                    nc.tensor.matmul(rb_ps[:], oner_t[:], recip_b[:],
                                     start=True, stop=True)
                    rb_sb = fpool.tile([128, QC], F32R, tag="f", name="f")
                    nc.vector.tensor_copy(rb_sb[:], rb_ps[:])
                    at = apool.tile([128, QC], BF16, tag="a", name="a")
                    nc.vector.tensor_mul(at[:], attn_sb[:], rb_sb[:])
                    a_sb[(j, h)] = at

                def emit_outproj_pairs(j, dst, qoff, heads):
                    # partial over this core's 256 head-features. m-tiles
                    # go in PAIRS into a two-bank psum with a single wide
                    # drain, alternating ACT/DVE (one ~1.2-1.4us drain per
                    # 4 matmuls keeps the drain off the PE's critical
                    # path — a per-m DVE drain at ~690ns paced the whole
                    # phase at ~790ns per 426ns of matmul). Stores split
                    # across the two otherwise-idle DMA queues.
                    for m0 in range(0, D // 128, 2):
                        ps = scoreps.tile([128, 2 * QC], F32, tag="ps",
                                          name="ps")
                        for i, m in enumerate((m0, m0 + 1)):
                            for hi, h in enumerate(heads):
                                nc.tensor.matmul(
                                    ps[:, i * QC:(i + 1) * QC],
                                    wo_tiles[h][:, m * 128:(m + 1) * 128],
                                    a_sb[(j, h)][:],
                                    start=(hi == 0),
                                    stop=(hi == len(heads) - 1))
                        o_sb = opool.tile([128, 2 * QC], BF16, tag="o",
                                          name="o")
                        if (m0 // 2) % 2 == 0:
                            nc.vector.tensor_copy(o_sb[:], ps[:])
                        else:
                            nc.scalar.copy(o_sb[:], ps[:])
                        for i, m in enumerate((m0, m0 + 1)):
                            q = nc.gpsimd if i == 0 else nc.sync
                            q.dma_start(
                                dst[m * 128:(m + 1) * 128, qoff:qoff + QC],
                                o_sb[:, i * QC:(i + 1) * QC])

                def emit_outproj(j):
                    emit_outproj_pairs(j, outT, b * S + j * QC, (0, 1))

                blocks = []
                for j in range(NCH):
                    emit_qkv_chunk(j)
                    for h in range(2):
                        blocks.append(emit_attn_block(j, h))
                        n = len(blocks)
                        if n >= 2:
                            emit_finalize(blocks[n - 2])
                        if n >= 3 and blocks[n - 3][1] == 1:
                            emit_outproj(blocks[n - 3][0])
                emit_finalize(blocks[-1])
                if b == B - 1:
                    # split the final chunk's out-proj by head: h0 depends
                    # only on finalize(j,h0) and runs before the h1 chain,
                    # halving the post-attention tail
                    jl = blocks[-1][0]
                    emit_outproj_pairs(jl, outT, b * S + jl * QC, (0,))
                    emit_outproj_pairs(jl, outT2, 0, (1,))
                else:
                    emit_outproj(blocks[-1][0])

    nc.compile()
    return nc


def make_inputs(cfg: Cfg, x, W_qkv, W_out):
    """Host-side sharding: returns in_maps (list of 8 dicts)."""
    B, S, D = cfg.B, cfg.S, cfg.D
    Dh = cfg.Dh
    xTa = np.ascontiguousarray(
        x.reshape(B * S, D).T).astype(ml_dtypes.bfloat16)  # [D, B*S]

    p = np.arange(128)[:, None]
    c = np.arange(128)[None, :]
    tri = np.where(p > c, -1e9, 0.0).astype(ml_dtypes.bfloat16)
    identm = np.eye(128, dtype=ml_dtypes.bfloat16)
    ones_col = np.ones((128, 1), dtype=ml_dtypes.bfloat16)
    ones_row = np.ones((1, 128), dtype=ml_dtypes.bfloat16)

    in_maps = []
    DHT = cfg.DHT
    for cidx in range(N_CORES):
        h0 = cfg.HPC * cidx
        wq = np.empty((D, 768), dtype=np.float32)          # [D, cols]
        for i, h in enumerate((h0, h0 + 1)):
            wq[:, (2 * i) * 128:(2 * i) * 128 + 128] = \
                W_qkv[0 * DHT + h * Dh: 0 * DHT + h * Dh + Dh, :].T   # Q_h
            wq[:, (2 * i + 1) * 128:(2 * i + 1) * 128 + 128] = \
                W_qkv[1 * DHT + h * Dh: 1 * DHT + h * Dh + Dh, :].T   # K_h
            wq[:, 512 + i * 128: 512 + (i + 1) * 128] = \
                W_qkv[2 * DHT + h * Dh: 2 * DHT + h * Dh + Dh, :].T   # V_h
        wo = np.ascontiguousarray(
            W_out[:, h0 * Dh:(h0 + cfg.HPC) * Dh].T).astype(
                ml_dtypes.bfloat16)
        in_maps.append({
            "xT": xTa,
            "wqkvT": wq.astype(ml_dtypes.bfloat16),
            "woutT": wo,
            "ident": identm,
            "tri": tri,
            "ones_col": ones_col,
            "ones_row": ones_row,
        })
    return in_maps


_CACHED = {}


def kernel(x, W_qkv, W_out, mask=None, **_ignored):
    cfg = Cfg(B=x.shape[0], S=x.shape[1], D=x.shape[2],
              n_heads=W_qkv.shape[0] // 384)
    key = (cfg.B, cfg.S, cfg.D)
    if key not in _CACHED:
        _CACHED[key] = build_kernel(cfg)
    nc = _CACHED[key]
    in_maps = make_inputs(cfg, np.asarray(x), np.asarray(W_qkv),
                          np.asarray(W_out))
    res = run_bass_kernel_spmd(nc, in_maps, list(range(N_CORES)))
    acc = res.results[0]["outT"].astype(np.float32)
    acc2 = res.results[0]["outT2"].astype(np.float32)
    for c in range(1, N_CORES):
        acc = acc + res.results[c]["outT"].astype(np.float32)
        acc2 = acc2 + res.results[c]["outT2"].astype(np.float32)
    acc[:, cfg.B * cfg.S - cfg.QC:] += acc2
    out = acc.T.reshape(cfg.B, cfg.S, cfg.D)
    return np.ascontiguousarray(out)
